# revision 34
# baseline (speedup 1.0000x reference)
"""Bass/Trainium2 kernel for framed 2-layer BiLSTM (nn_BLSTM).

Data-parallel over the 80 framed sequences: 10 per core on 8 NeuronCores.
All matmuls in bf16 (f32 PSUM accumulation). The recurrence runs in a
TRANSPOSED layout [channels(partitions) x sequences(free)]: per step the
xw slice is injected into PSUM via PE transposes (start of the accumulate
group) and the Wh contribution streams as 128x128-stationary matmuls with
N=10 moving columns. Hidden states land directly in resident transposed
h-buffers that feed both the next step's matmuls and the next layer's
input projection as stationary operands.
"""
import sys
import numpy as np

sys.path.insert(0, "/opt/trn_rl_repo")

import ml_dtypes  # noqa: E402
import concourse.bass as bass  # noqa: E402
import concourse.mybir as mybir  # noqa: E402
from concourse import bacc  # noqa: E402
from concourse.tile import TileContext  # noqa: E402
from concourse.masks import make_identity  # noqa: E402
from concourse.bass_utils import run_bass_kernel_spmd  # noqa: E402

F32 = mybir.dt.float32
BF16 = mybir.dt.bfloat16
FP8 = mybir.dt.float8e4
BF16_NP = ml_dtypes.bfloat16
FP8_NP = ml_dtypes.float8_e4m3
DR = mybir.MatmulPerfMode.DoubleRow

DIM = 768
H = 768
G = 4 * H            # 3072, gate order reordered to [i, f, o, g]
B, T = 4, 2000
WIDTH, STRIDE = 200, 100
NFR = 20             # frames per batch element
NSEQ = B * NFR       # 80
NCORES = 8
SEQ_PC = NSEQ // NCORES   # 10
ROWS = SEQ_PC * WIDTH     # 2000 rows per core, row = t*10 + s (t-major)
MT = (ROWS + 127) // 128  # 16 row m-tiles (last has 80 rows)
CH_STEPS = 10             # timesteps per xw chunk DMA
NCH = WIDTH // CH_STEPS   # 20 chunks

SIG = mybir.ActivationFunctionType.Sigmoid
TANH = mybir.ActivationFunctionType.Tanh
MUL = mybir.AluOpType.mult
ADD = mybir.AluOpType.add

_CACHE = {}


def _mrows(m):
    return min(128, ROWS - m * 128)


def _build_program():
    nc = bacc.Bacc("TRN2", target_bir_lowering=False, debug=False,
                   num_devices=NCORES)

    xT_d = nc.declare_dram_parameter("xT", [DIM, ROWS], FP8, isOutput=False)
    wx0_d = nc.declare_dram_parameter("wx0", [2, DIM, G], FP8, isOutput=False)
    wh0_d = nc.declare_dram_parameter("wh0", [2, H, G], BF16, isOutput=False)
    b0_d = nc.declare_dram_parameter("b0", [2, 128, 24], F32, isOutput=False)
    wx1_d = nc.declare_dram_parameter("wx1", [2, 2 * H, G], FP8,
                                      isOutput=False)
    wh1_d = nc.declare_dram_parameter("wh1", [2, H, G], BF16, isOutput=False)
    b1_d = nc.declare_dram_parameter("b1", [2, 128, 24], F32, isOutput=False)
    linw_d = nc.declare_dram_parameter("linw", [2 * H, DIM], BF16,
                                       isOutput=False)
    linb_d = nc.declare_dram_parameter("linb", [DIM], F32, isOutput=False)
    out_d = nc.declare_dram_parameter("out", [ROWS, DIM], F32, isOutput=True)

    # xw stored transposed: [dir, gate m-tile, partition(gate%128), row]
    xw0_d = nc.dram_tensor("xw0", [2, 24, 128, ROWS], BF16)
    xw1_d = nc.dram_tensor("xw1", [2, 24, 128, ROWS], BF16)

    with TileContext(nc) as tc:
        with tc.tile_pool(name="const", bufs=1) as constp:
            identb = constp.tile([128, 128], BF16)
            make_identity(nc, identb[:])
            ones = constp.tile([1, 128], F32)
            nc.vector.memset(ones[:], 1.0)

            # ---- batched input projection, transposed output ----
            # xwT[d, m, p, row] = sum_k rhs_fn(row)[k] * Wx[k, m*128+p] + b
            # fp8 DoubleRow: each matmul consumes two 128-row K-tiles via
            # [128, 2, *] APs on both operands.
            def proj(kt, rhs_fn, wx_dram, b_dram, xw_dram, mid=None,
                     interleave=None):
                k2t = kt // 2
                for d in range(2):
                    if d == 1 and mid is not None:
                        mid()
                    with tc.tile_pool(name="wxp", bufs=1) as wxp, \
                         tc.tile_pool(name="bbp", bufs=1) as bbp, \
                         tc.tile_pool(name="pp", bufs=3, space="PSUM") as pp, \
                         tc.tile_pool(name="xo", bufs=3) as xop:
                        wx_sb = wxp.tile([128, kt, G], FP8)
                        for k in range(kt):
                            nc.sync.dma_start(
                                wx_sb[:, k],
                                wx_dram[d, k * 128:(k + 1) * 128, :])
                            if interleave is not None:
                                interleave(d, k)
                        bT = bbp.tile([128, 24], F32)
                        nc.sync.dma_start(bT[:], b_dram[d])
                        for m in range(24):
                            for r in range(4):
                                rs = slice(r * 500, (r + 1) * 500)
                                ps = pp.tile([128, 500], F32, tag="pp")
                                for k2 in range(k2t):
                                    nc.tensor.matmul(
                                        ps[:],
                                        wx_sb[:, 2 * k2:2 * k2 + 2,
                                              m * 128:(m + 1) * 128],
                                        rhs_fn(d, k2, rs),
                                        start=(k2 == 0), stop=(k2 == k2t - 1),
                                        perf_mode=DR)
                                xo = xop.tile([128, 500], BF16, tag="xo")
                                if (m * 4 + r) % 2 == 0:
                                    nc.vector.tensor_scalar(
                                        xo[:], ps[:], bT[:, m:m + 1], None,
                                        ADD)
                                else:
                                    nc.scalar.add(xo[:], ps[:],
                                                  bT[:, m:m + 1])
                                nc.sync.dma_start(xw_dram[d, m, :, rs], xo[:])

            # ---- recurrence, both dirs interleaved, transposed layout ----
            # Gates split into two PSUM halves: [i,f] (m-tiles 0-11) and
            # [o,g] (m-tiles 12-23), so the sigmoid chain of a step starts
            # after only half its matmuls and hides under the rest.
            def recur(whp, wh_sb, xw_dram, hT, h8=None):
                with tc.tile_pool(name="stp", bufs=1) as stp, \
                     tc.tile_pool(name="xcp", bufs=2) as xcp, \
                     tc.tile_pool(name="gpp", bufs=2, space="PSUM") as gpp, \
                     tc.tile_pool(name="sgp", bufs=2) as sgp:
                    c = [stp.tile([128, 60], F32, name=f"c{d}")
                         for d in range(2)]
                    for d in range(2):
                        nc.vector.memset(c[d][:], 0.0)

                    for ch in range(NCH):
                        xc = []
                        for d in range(2):
                            cs = ch * 100 if d == 0 else 1900 - ch * 100
                            xct = xcp.tile([128, 24, 100], BF16,
                                           tag=f"xc{d}", name=f"xc{d}")
                            nc.sync.dma_start(
                                xct[:],
                                xw_dram[d, :, :, cs:cs + 100].rearrange(
                                    "m p c -> p m c"))
                            xc.append(xct)
                        for tl in range(CH_STEPS):
                            t = ch * CH_STEPS + tl
                            first = (t == 0)
                            sg, tcs = [], []
                            for d in range(2):
                                tt = t if d == 0 else WIDTH - 1 - t
                                lc = tl * 10 if d == 0 else (9 - tl) * 10
                                pv = (t - 1) * 10 if d == 0 else (tt + 1) * 10
                                gif = gpp.tile([128, 120], F32,
                                               tag=f"gi{d}", name=f"gi{d}")
                                gog = gpp.tile([128, 120], F32,
                                               tag=f"go{d}", name=f"go{d}")
                                for m in range(24):
                                    gg = gif if m < 12 else gog
                                    ms = slice((m % 12) * 10,
                                               (m % 12) * 10 + 10)
                                    nc.tensor.matmul(
                                        gg[:, ms],
                                        identb[:],
                                        xc[d][:, m, lc:lc + 10],
                                        start=True, stop=first)
                                    if not first:
                                        for k in range(6):
                                            nc.tensor.matmul(
                                                gg[:, ms],
                                                wh_sb[:, d, k,
                                                      m * 128:(m + 1) * 128],
                                                hT[d][:, k, pv:pv + 10],
                                                start=False, stop=(k == 5))
                                sgif = sgp.tile([128, 120], F32,
                                                tag=f"si{d}", name=f"si{d}")
                                tog = sgp.tile([128, 120], F32,
                                               tag=f"to{d}", name=f"to{d}")
                                tmm = sgp.tile([128, 60], F32,
                                               tag=f"tm{d}", name=f"tm{d}")
                                sg.append((sgif, tog))
                                tcs.append(sgp.tile([128, 60], F32,
                                                    tag=f"tc{d}",
                                                    name=f"tc{d}"))
                                nc.scalar.activation(sgif[:], gif[:], SIG)
                                # o prescaled by 0.5 host-side: one tanh
                                # covers [o', g]; sig(o) = (tanh(o') + 1)/2
                                # and the /2 is folded into h-consumer
                                # weights (h is stored as 2h).
                                nc.scalar.activation(tog[:], gog[:], TANH)
                                nc.gpsimd.tensor_tensor(
                                    tmm[:], sgif[:, 0:60], tog[:, 60:120],
                                    MUL)
                                nc.vector.tensor_tensor(
                                    c[d][:], c[d][:], sgif[:, 60:120], MUL)
                                nc.vector.tensor_tensor(
                                    c[d][:], c[d][:], tmm[:], ADD)
                            for d in range(2):
                                tt = t if d == 0 else WIDTH - 1 - t
                                nc.scalar.activation(tcs[d][:], c[d][:], TANH)
                                # h2 = (tanh(o') + 1) * tanh(c) == 2h
                                nc.vector.scalar_tensor_tensor(
                                    hT[d][:, :, tt * 10:(tt + 1) * 10],
                                    sg[d][1][:, 0:60].rearrange(
                                        "p (a b) -> p a b", b=10),
                                    1.0,
                                    tcs[d][:].rearrange(
                                        "p (a b) -> p a b", b=10),
                                    ADD, MUL)
                            if h8 is not None:
                                for d in range(2):
                                    tt = t if d == 0 else WIDTH - 1 - t
                                    nc.vector.scalar_tensor_tensor(
                                        h8[d][:, :, tt * 10:(tt + 1) * 10],
                                        sg[d][1][:, 0:60].rearrange(
                                            "p (a b) -> p a b", b=10),
                                        1.0,
                                        tcs[d][:].rearrange(
                                            "p (a b) -> p a b", b=10),
                                        ADD, MUL)

            # ---- final linear: y = h1cat @ linW + linb ----
            def linear(hT1, lwp, lw, lbsb):
                with tc.tile_pool(name="lpp", bufs=3, space="PSUM") as lpp, \
                     tc.tile_pool(name="lop", bufs=2) as lop:
                    lbb = lwp.tile([128, DIM], F32)
                    for n in range(2):
                        ns = slice(n * 384, (n + 1) * 384)
                        bps = lpp.tile([128, 384], F32, tag="lp")
                        nc.tensor.matmul(bps[:], ones[:], lbsb[:, ns],
                                         start=True, stop=True)
                        nc.vector.tensor_copy(lbb[:, ns], bps[:])
                    for m in range(MT):
                        mr = _mrows(m)
                        lo = lop.tile([128, DIM], F32, tag="lo")
                        for n in range(2):
                            ns = slice(n * 384, (n + 1) * 384)
                            ps = lpp.tile([mr, 384], F32, tag="lp")
                            for k in range(12):
                                ht = hT1[0] if k < 6 else hT1[1]
                                kk = k if k < 6 else k - 6
                                nc.tensor.matmul(
                                    ps[:],
                                    ht[:, kk, m * 128:m * 128 + mr],
                                    lw[:, k, ns],
                                    start=(k == 0), stop=(k == 11))
                            nc.vector.tensor_tensor(
                                lo[:mr, ns], ps[:], lbb[:mr, ns], ADD)
                        nc.sync.dma_start(out_d[m * 128:m * 128 + mr, :],
                                          lo[:mr])

            def load_wh(whp, wh_dram, name):
                wh_sb = whp.tile([128, 2, 6, G], BF16, name=name)
                for d in range(2):
                    for k in range(6):
                        nc.sync.dma_start(
                            wh_sb[:, d, k],
                            wh_dram[d, k * 128:(k + 1) * 128, :])
                return wh_sb

            # h0/h1 share 2 slots: h1 reuses h0's space after proj1.
            with tc.tile_pool(name="hbp", bufs=2) as hbp, \
                 tc.tile_pool(name="h8p", bufs=1) as h8p:
                hT0 = [hbp.tile([128, 6, ROWS], BF16, tag="hb",
                                name=f"h0{d}") for d in range(2)]
                hT1 = [hbp.tile([128, 6, ROWS], BF16, tag="hb",
                                name=f"h1{d}") for d in range(2)]
                h08 = [h8p.tile([128, 6, ROWS], FP8, name=f"h08{d}")
                       for d in range(2)]
                with tc.tile_pool(name="whp0", bufs=1) as whp0:
                    wh0_box = []
                    with tc.tile_pool(name="xtp", bufs=1) as xtp:
                        xT_sb = xtp.tile([128, 6, ROWS], FP8)
                        proj(6,
                             lambda d, k2, rs:
                             xT_sb[:, 2 * k2:2 * k2 + 2, rs],
                             wx0_d, b0_d, xw0_d,
                             mid=lambda: wh0_box.append(
                                 load_wh(whp0, wh0_d, "wh0")),
                             interleave=lambda d, k:
                             nc.sync.dma_start(
                                 xT_sb[:, k],
                                 xT_d[k * 128:(k + 1) * 128, :])
                             if d == 0 else None)
                    recur(whp0, wh0_box[0], xw0_d, hT0, h8=h08)
                with tc.tile_pool(name="whp1", bufs=1) as whp1:
                    wh1_box = []
                    proj(12,
                         lambda d, k2, rs:
                         h08[0][:, 2 * k2:2 * k2 + 2, rs] if k2 < 3
                         else h08[1][:, 2 * k2 - 6:2 * k2 - 4, rs],
                         wx1_d, b1_d, xw1_d,
                         mid=lambda: wh1_box.append(
                             load_wh(whp1, wh1_d, "wh1")))
                    with tc.tile_pool(name="lwp", bufs=1) as lwp:
                        lw = lwp.tile([128, 12, DIM], BF16)
                        for k in range(12):
                            nc.sync.dma_start(
                                lw[:, k], linw_d[k * 128:(k + 1) * 128, :])
                        lbsb = lwp.tile([1, DIM], F32)
                        nc.sync.dma_start(lbsb[:], linb_d[None, :])
                        recur(whp1, wh1_box[0], xw1_d, hT1)
                        linear(hT1, lwp, lw, lbsb)

    nc.compile()
    return nc


def _reorder_gates(w):
    """[i f g o] -> [i f o g] along last axis (size 4H)."""
    i, f, g, o = np.split(w, 4, axis=-1)
    return np.concatenate([i, f, o, g], axis=-1)


def kernel(x, Wx0f, Wh0f, b0f, Wx0b, Wh0b, b0b,
           Wx1f, Wh1f, b1f, Wx1b, Wh1b, b1b, lin_W, lin_b):
    x = np.asarray(x, dtype=np.float32)
    # frame: (B, C, T) -> (NSEQ, DIM, WIDTH)
    tgt = (NFR - 1) * STRIDE + WIDTH
    xp = np.zeros((B, DIM, tgt), dtype=np.float32)
    xp[:, :, :T] = x
    frames = np.stack([xp[:, :, i:i + WIDTH]
                       for i in range(0, tgt - WIDTH + 1, STRIDE)], axis=1)
    xf = frames.reshape(NSEQ, DIM, WIDTH)

    # Gate order [i,f,o,g]; o-columns prescaled 0.5 so sig(o)=(tanh(o')+1)/2;
    # h is stored as 2h, so all h-consuming weight rows absorb a 0.5.
    def prepw(wf, wb, dt, row_scale):
        def one(w):
            w = _reorder_gates(np.asarray(w, np.float32)).copy()
            w[:, 2 * H:3 * H] *= 0.5
            return w * row_scale
        return np.ascontiguousarray(np.stack(
            [one(wf), one(wb)])).astype(dt)

    def prepb(bf, bb_):
        # transposed bias: [dir, partition(gate%128), gate m-tile]
        def one(b):
            b = _reorder_gates(np.asarray(b, np.float32)).copy()
            b[2 * H:3 * H] *= 0.5
            return b.reshape(24, 128).T
        return np.ascontiguousarray(np.stack([one(bf), one(bb_)]))

    wx0 = prepw(Wx0f, Wx0b, FP8_NP, 1.0)
    wh0 = prepw(Wh0f, Wh0b, BF16_NP, 0.5)
    b0 = prepb(b0f, b0b)
    wx1 = prepw(Wx1f, Wx1b, FP8_NP, 0.5)
    wh1 = prepw(Wh1f, Wh1b, BF16_NP, 0.5)
    b1 = prepb(b1f, b1b)
    linw = np.ascontiguousarray(
        np.asarray(lin_W, np.float32) * 0.5).astype(BF16_NP)
    linb = np.ascontiguousarray(np.asarray(lin_b, np.float32))

    if "nc" not in _CACHE:
        _CACHE["nc"] = _build_program()
    nc = _CACHE["nc"]

    in_maps = []
    for cc in range(NCORES):
        shard = xf[cc * SEQ_PC:(cc + 1) * SEQ_PC]       # (10, 768, 200)
        xT = shard.transpose(1, 2, 0).reshape(DIM, ROWS)  # col = t*10 + s
        in_maps.append({"xT": np.ascontiguousarray(xT).astype(FP8_NP),
                        "wx0": wx0, "wh0": wh0, "b0": b0,
                        "wx1": wx1, "wh1": wh1, "b1": b1,
                        "linw": linw, "linb": linb})
    _CACHE["in_maps"] = in_maps

    res = run_bass_kernel_spmd(nc, in_maps, list(range(NCORES)))
    outs = [np.asarray(res.results[cc]["out"], np.float32)
            .reshape(WIDTH, SEQ_PC, DIM).transpose(1, 0, 2)
            for cc in range(NCORES)]                     # (10, 200, 768)
    y = np.concatenate(outs, axis=0)                     # (80, 200, 768)
    y = y.transpose(0, 2, 1).reshape(B, NFR, DIM, WIDTH)

    limit = STRIDE // 2
    parts = [y[:, 0, :, :-limit]]
    for k in range(1, NFR - 1):
        parts.append(y[:, k, :, limit:-limit])
    parts.append(y[:, NFR - 1, :, limit:])
    yc = np.concatenate(parts, axis=-1)[:, :, :T]        # (4, 768, 2000)
    return (yc + x).astype(np.float32)


# revision 35
# speedup vs baseline: 1.0326x; 1.0326x over previous
"""Bass/Trainium2 kernel for framed 2-layer BiLSTM (nn_BLSTM).

Data-parallel over the 80 framed sequences: 10 per core on 8 NeuronCores.
All matmuls in bf16 (f32 PSUM accumulation). The recurrence runs in a
TRANSPOSED layout [channels(partitions) x sequences(free)]: per step the
xw slice is injected into PSUM via PE transposes (start of the accumulate
group) and the Wh contribution streams as 128x128-stationary matmuls with
N=10 moving columns. Hidden states land directly in resident transposed
h-buffers that feed both the next step's matmuls and the next layer's
input projection as stationary operands.
"""
import sys
import numpy as np

sys.path.insert(0, "/opt/trn_rl_repo")

import ml_dtypes  # noqa: E402
import concourse.bass as bass  # noqa: E402
import concourse.mybir as mybir  # noqa: E402
from concourse import bacc  # noqa: E402
from concourse.tile import TileContext  # noqa: E402
from concourse.masks import make_identity  # noqa: E402
from concourse.bass_utils import run_bass_kernel_spmd  # noqa: E402

F32 = mybir.dt.float32
BF16 = mybir.dt.bfloat16
FP8 = mybir.dt.float8e4
BF16_NP = ml_dtypes.bfloat16
FP8_NP = ml_dtypes.float8_e4m3
DR = mybir.MatmulPerfMode.DoubleRow

DIM = 768
H = 768
G = 4 * H            # 3072, gate order reordered to [i, f, o, g]
B, T = 4, 2000
WIDTH, STRIDE = 200, 100
NFR = 20             # frames per batch element
NSEQ = B * NFR       # 80
NCORES = 8
SEQ_PC = NSEQ // NCORES   # 10
ROWS = SEQ_PC * WIDTH     # 2000 rows per core, row = t*10 + s (t-major)
MT = (ROWS + 127) // 128  # 16 row m-tiles (last has 80 rows)
CH_STEPS = 10             # timesteps per xw chunk DMA
NCH = WIDTH // CH_STEPS   # 20 chunks

SIG = mybir.ActivationFunctionType.Sigmoid
TANH = mybir.ActivationFunctionType.Tanh
MUL = mybir.AluOpType.mult
ADD = mybir.AluOpType.add

_CACHE = {}


def _mrows(m):
    return min(128, ROWS - m * 128)


def _build_program():
    nc = bacc.Bacc("TRN2", target_bir_lowering=False, debug=False,
                   num_devices=NCORES)

    xT_d = nc.declare_dram_parameter("xT", [DIM, ROWS], FP8, isOutput=False)
    wx0_d = nc.declare_dram_parameter("wx0", [2, DIM, G], FP8, isOutput=False)
    wh0_d = nc.declare_dram_parameter("wh0", [2, H, G], BF16, isOutput=False)
    b0_d = nc.declare_dram_parameter("b0", [2, 128, 24], F32, isOutput=False)
    wx1_d = nc.declare_dram_parameter("wx1", [2, 2 * H, G], FP8,
                                      isOutput=False)
    wh1_d = nc.declare_dram_parameter("wh1", [2, H, G], BF16, isOutput=False)
    b1_d = nc.declare_dram_parameter("b1", [2, 128, 24], F32, isOutput=False)
    linw_d = nc.declare_dram_parameter("linw", [2 * H, DIM], BF16,
                                       isOutput=False)
    linb_d = nc.declare_dram_parameter("linb", [DIM], F32, isOutput=False)
    out_d = nc.declare_dram_parameter("out", [ROWS, DIM], F32, isOutput=True)

    # xw stored transposed: [dir, gate m-tile, partition(gate%128), row]
    xw0_d = nc.dram_tensor("xw0", [2, 24, 128, ROWS], BF16)
    xw1_d = nc.dram_tensor("xw1", [2, 24, 128, ROWS], BF16)

    with TileContext(nc) as tc:
        with tc.tile_pool(name="const", bufs=1) as constp:
            identb = constp.tile([128, 128], BF16)
            make_identity(nc, identb[:])
            ones = constp.tile([1, 128], F32)
            nc.vector.memset(ones[:], 1.0)

            # ---- batched input projection, transposed output ----
            # xwT[d, m, p, row] = sum_k rhs_fn(row)[k] * Wx[k, m*128+p] + b
            # fp8 DoubleRow: each matmul consumes two 128-row K-tiles via
            # [128, 2, *] APs on both operands.
            def proj(kt, rhs_fn, wx_dram, b_dram, xw_dram, mid=None,
                     interleave=None):
                k2t = kt // 2
                for d in range(2):
                    if d == 1 and mid is not None:
                        mid()
                    with tc.tile_pool(name="wxp", bufs=1) as wxp, \
                         tc.tile_pool(name="bbp", bufs=1) as bbp, \
                         tc.tile_pool(name="pp", bufs=3, space="PSUM") as pp, \
                         tc.tile_pool(name="xo", bufs=3) as xop:
                        wx_sb = wxp.tile([128, kt, G], FP8)
                        for k in range(kt):
                            nc.sync.dma_start(
                                wx_sb[:, k],
                                wx_dram[d, k * 128:(k + 1) * 128, :])
                            if interleave is not None:
                                interleave(d, k)
                        bT = bbp.tile([128, 24], F32)
                        nc.sync.dma_start(bT[:], b_dram[d])
                        for m in range(24):
                            for r in range(4):
                                rs = slice(r * 500, (r + 1) * 500)
                                ps = pp.tile([128, 500], F32, tag="pp")
                                for k2 in range(k2t):
                                    nc.tensor.matmul(
                                        ps[:],
                                        wx_sb[:, 2 * k2:2 * k2 + 2,
                                              m * 128:(m + 1) * 128],
                                        rhs_fn(d, k2, rs),
                                        start=(k2 == 0), stop=(k2 == k2t - 1),
                                        perf_mode=DR)
                                xo = xop.tile([128, 500], BF16, tag="xo")
                                if (m * 4 + r) % 2 == 0:
                                    nc.vector.tensor_scalar(
                                        xo[:], ps[:], bT[:, m:m + 1], None,
                                        ADD)
                                else:
                                    nc.scalar.add(xo[:], ps[:],
                                                  bT[:, m:m + 1])
                                nc.sync.dma_start(xw_dram[d, m, :, rs], xo[:])

            # ---- recurrence, both dirs interleaved, transposed layout ----
            # Gates split into two PSUM halves: [i,f] (m-tiles 0-11) and
            # [o,g] (m-tiles 12-23), so the sigmoid chain of a step starts
            # after only half its matmuls and hides under the rest.
            def recur(whp, wh_sb, xw_dram, hT, h8=None):
                with tc.tile_pool(name="stp", bufs=1) as stp, \
                     tc.tile_pool(name="xcp", bufs=2) as xcp, \
                     tc.tile_pool(name="gpp", bufs=2, space="PSUM") as gpp, \
                     tc.tile_pool(name="sgp", bufs=2) as sgp:
                    c = [stp.tile([128, 60], F32, name=f"c{d}")
                         for d in range(2)]
                    for d in range(2):
                        nc.vector.memset(c[d][:], 0.0)

                    for ch in range(NCH):
                        xc = []
                        for d in range(2):
                            cs = ch * 100 if d == 0 else 1900 - ch * 100
                            xct = xcp.tile([128, 24, 100], BF16,
                                           tag=f"xc{d}", name=f"xc{d}")
                            nc.sync.dma_start(
                                xct[:],
                                xw_dram[d, :, :, cs:cs + 100].rearrange(
                                    "m p c -> p m c"))
                            xc.append(xct)
                        for tl in range(CH_STEPS):
                            t = ch * CH_STEPS + tl
                            first = (t == 0)
                            sg, tcs = [], []
                            for d in range(2):
                                tt = t if d == 0 else WIDTH - 1 - t
                                lc = tl * 10 if d == 0 else (9 - tl) * 10
                                pv = (t - 1) * 10 if d == 0 else (tt + 1) * 10
                                gif = gpp.tile([128, 120], F32,
                                               tag=f"gi{d}", name=f"gi{d}")
                                gog = gpp.tile([128, 120], F32,
                                               tag=f"go{d}", name=f"go{d}")
                                for m in range(24):
                                    gg = gif if m < 12 else gog
                                    ms = slice((m % 12) * 10,
                                               (m % 12) * 10 + 10)
                                    nc.tensor.matmul(
                                        gg[:, ms],
                                        identb[:],
                                        xc[d][:, m, lc:lc + 10],
                                        start=True, stop=first)
                                    if not first:
                                        for k in range(6):
                                            nc.tensor.matmul(
                                                gg[:, ms],
                                                wh_sb[:, d, k,
                                                      m * 128:(m + 1) * 128],
                                                hT[d][:, k, pv:pv + 10],
                                                start=False, stop=(k == 5))
                                sgif = sgp.tile([128, 120], F32,
                                                tag=f"si{d}", name=f"si{d}")
                                tgg = sgp.tile([128, 60], F32,
                                               tag=f"tg{d}", name=f"tg{d}")
                                sgo = sgp.tile([128, 60], F32,
                                               tag=f"so{d}", name=f"so{d}")
                                tmm = sgp.tile([128, 60], F32,
                                               tag=f"tm{d}", name=f"tm{d}")
                                sg.append((sgif, sgo))
                                tcs.append(sgp.tile([128, 60], F32,
                                                    tag=f"tc{d}",
                                                    name=f"tc{d}"))
                                nc.scalar.activation(sgif[:], gif[:], SIG)
                                nc.scalar.activation(tgg[:], gog[:, 60:120],
                                                     TANH)
                                nc.scalar.activation(sgo[:], gog[:, 0:60],
                                                     SIG)
                                nc.gpsimd.tensor_tensor(
                                    tmm[:], sgif[:, 0:60], tgg[:], MUL)
                                nc.vector.tensor_tensor(
                                    c[d][:], c[d][:], sgif[:, 60:120], MUL)
                                nc.vector.tensor_tensor(
                                    c[d][:], c[d][:], tmm[:], ADD)
                            for d in range(2):
                                tt = t if d == 0 else WIDTH - 1 - t
                                nc.scalar.activation(tcs[d][:], c[d][:], TANH)
                                nc.vector.tensor_tensor(
                                    hT[d][:, :, tt * 10:(tt + 1) * 10],
                                    sg[d][1][:].rearrange(
                                        "p (a b) -> p a b", b=10),
                                    tcs[d][:].rearrange(
                                        "p (a b) -> p a b", b=10),
                                    MUL)
                            if h8 is not None:
                                for d in range(2):
                                    tt = t if d == 0 else WIDTH - 1 - t
                                    nc.gpsimd.tensor_tensor(
                                        h8[d][:, :, tt * 10:(tt + 1) * 10],
                                        sg[d][1][:].rearrange(
                                            "p (a b) -> p a b", b=10),
                                        tcs[d][:].rearrange(
                                            "p (a b) -> p a b", b=10),
                                        MUL)

            # ---- final linear: y = h1cat @ linW + linb ----
            def linear(hT1, lwp, lw, lbsb):
                with tc.tile_pool(name="lpp", bufs=3, space="PSUM") as lpp, \
                     tc.tile_pool(name="lop", bufs=2) as lop:
                    lbb = lwp.tile([128, DIM], F32)
                    for n in range(2):
                        ns = slice(n * 384, (n + 1) * 384)
                        bps = lpp.tile([128, 384], F32, tag="lp")
                        nc.tensor.matmul(bps[:], ones[:], lbsb[:, ns],
                                         start=True, stop=True)
                        nc.vector.tensor_copy(lbb[:, ns], bps[:])
                    for m in range(MT):
                        mr = _mrows(m)
                        lo = lop.tile([128, DIM], F32, tag="lo")
                        for n in range(2):
                            ns = slice(n * 384, (n + 1) * 384)
                            ps = lpp.tile([mr, 384], F32, tag="lp")
                            for k in range(12):
                                ht = hT1[0] if k < 6 else hT1[1]
                                kk = k if k < 6 else k - 6
                                nc.tensor.matmul(
                                    ps[:],
                                    ht[:, kk, m * 128:m * 128 + mr],
                                    lw[:, k, ns],
                                    start=(k == 0), stop=(k == 11))
                            nc.vector.tensor_tensor(
                                lo[:mr, ns], ps[:], lbb[:mr, ns], ADD)
                        nc.sync.dma_start(out_d[m * 128:m * 128 + mr, :],
                                          lo[:mr])

            def load_wh(whp, wh_dram, name):
                wh_sb = whp.tile([128, 2, 6, G], BF16, name=name)
                for d in range(2):
                    for k in range(6):
                        nc.sync.dma_start(
                            wh_sb[:, d, k],
                            wh_dram[d, k * 128:(k + 1) * 128, :])
                return wh_sb

            # h0/h1 share 2 slots: h1 reuses h0's space after proj1.
            with tc.tile_pool(name="hbp", bufs=2) as hbp, \
                 tc.tile_pool(name="h8p", bufs=1) as h8p:
                hT0 = [hbp.tile([128, 6, ROWS], BF16, tag="hb",
                                name=f"h0{d}") for d in range(2)]
                hT1 = [hbp.tile([128, 6, ROWS], BF16, tag="hb",
                                name=f"h1{d}") for d in range(2)]
                h08 = [h8p.tile([128, 6, ROWS], FP8, name=f"h08{d}")
                       for d in range(2)]
                with tc.tile_pool(name="whp0", bufs=1) as whp0:
                    wh0_box = []
                    with tc.tile_pool(name="xtp", bufs=1) as xtp:
                        xT_sb = xtp.tile([128, 6, ROWS], FP8)
                        proj(6,
                             lambda d, k2, rs:
                             xT_sb[:, 2 * k2:2 * k2 + 2, rs],
                             wx0_d, b0_d, xw0_d,
                             mid=lambda: wh0_box.append(
                                 load_wh(whp0, wh0_d, "wh0")),
                             interleave=lambda d, k:
                             nc.sync.dma_start(
                                 xT_sb[:, k],
                                 xT_d[k * 128:(k + 1) * 128, :])
                             if d == 0 else None)
                    recur(whp0, wh0_box[0], xw0_d, hT0, h8=h08)
                with tc.tile_pool(name="whp1", bufs=1) as whp1:
                    wh1_box = []
                    proj(12,
                         lambda d, k2, rs:
                         h08[0][:, 2 * k2:2 * k2 + 2, rs] if k2 < 3
                         else h08[1][:, 2 * k2 - 6:2 * k2 - 4, rs],
                         wx1_d, b1_d, xw1_d,
                         mid=lambda: wh1_box.append(
                             load_wh(whp1, wh1_d, "wh1")))
                    with tc.tile_pool(name="lwp", bufs=1) as lwp:
                        lw = lwp.tile([128, 12, DIM], BF16)
                        for k in range(12):
                            nc.sync.dma_start(
                                lw[:, k], linw_d[k * 128:(k + 1) * 128, :])
                        lbsb = lwp.tile([1, DIM], F32)
                        nc.sync.dma_start(lbsb[:], linb_d[None, :])
                        recur(whp1, wh1_box[0], xw1_d, hT1)
                        linear(hT1, lwp, lw, lbsb)

    nc.compile()
    return nc


def _reorder_gates(w):
    """[i f g o] -> [i f o g] along last axis (size 4H)."""
    i, f, g, o = np.split(w, 4, axis=-1)
    return np.concatenate([i, f, o, g], axis=-1)


def kernel(x, Wx0f, Wh0f, b0f, Wx0b, Wh0b, b0b,
           Wx1f, Wh1f, b1f, Wx1b, Wh1b, b1b, lin_W, lin_b):
    x = np.asarray(x, dtype=np.float32)
    # frame: (B, C, T) -> (NSEQ, DIM, WIDTH)
    tgt = (NFR - 1) * STRIDE + WIDTH
    xp = np.zeros((B, DIM, tgt), dtype=np.float32)
    xp[:, :, :T] = x
    frames = np.stack([xp[:, :, i:i + WIDTH]
                       for i in range(0, tgt - WIDTH + 1, STRIDE)], axis=1)
    xf = frames.reshape(NSEQ, DIM, WIDTH)

    def prepw(wf, wb, dt=BF16_NP):
        return np.ascontiguousarray(np.stack(
            [_reorder_gates(np.asarray(wf, np.float32)),
             _reorder_gates(np.asarray(wb, np.float32))])).astype(dt)

    def prepb(bf, bb_):
        # transposed bias: [dir, partition(gate%128), gate m-tile]
        return np.ascontiguousarray(np.stack(
            [_reorder_gates(np.asarray(bf, np.float32)).reshape(24, 128).T,
             _reorder_gates(np.asarray(bb_, np.float32)).reshape(24, 128).T]))

    wx0 = prepw(Wx0f, Wx0b, FP8_NP)
    wh0 = prepw(Wh0f, Wh0b)
    b0 = prepb(b0f, b0b)
    wx1 = prepw(Wx1f, Wx1b, FP8_NP)
    wh1 = prepw(Wh1f, Wh1b)
    b1 = prepb(b1f, b1b)
    linw = np.ascontiguousarray(np.asarray(lin_W, np.float32)).astype(BF16_NP)
    linb = np.ascontiguousarray(np.asarray(lin_b, np.float32))

    if "nc" not in _CACHE:
        _CACHE["nc"] = _build_program()
    nc = _CACHE["nc"]

    in_maps = []
    for cc in range(NCORES):
        shard = xf[cc * SEQ_PC:(cc + 1) * SEQ_PC]       # (10, 768, 200)
        xT = shard.transpose(1, 2, 0).reshape(DIM, ROWS)  # col = t*10 + s
        in_maps.append({"xT": np.ascontiguousarray(xT).astype(FP8_NP),
                        "wx0": wx0, "wh0": wh0, "b0": b0,
                        "wx1": wx1, "wh1": wh1, "b1": b1,
                        "linw": linw, "linb": linb})
    _CACHE["in_maps"] = in_maps

    res = run_bass_kernel_spmd(nc, in_maps, list(range(NCORES)))
    outs = [np.asarray(res.results[cc]["out"], np.float32)
            .reshape(WIDTH, SEQ_PC, DIM).transpose(1, 0, 2)
            for cc in range(NCORES)]                     # (10, 200, 768)
    y = np.concatenate(outs, axis=0)                     # (80, 200, 768)
    y = y.transpose(0, 2, 1).reshape(B, NFR, DIM, WIDTH)

    limit = STRIDE // 2
    parts = [y[:, 0, :, :-limit]]
    for k in range(1, NFR - 1):
        parts.append(y[:, k, :, limit:-limit])
    parts.append(y[:, NFR - 1, :, limit:])
    yc = np.concatenate(parts, axis=-1)[:, :, :T]        # (4, 768, 2000)
    return (yc + x).astype(np.float32)


# revision 37
# speedup vs baseline: 1.1797x; 1.1424x over previous
"""Bass/Trainium2 kernel for framed 2-layer BiLSTM (nn_BLSTM).

Data-parallel over the 80 framed sequences: 10 per core on 8 NeuronCores.
All matmuls in bf16 (f32 PSUM accumulation). The recurrence runs in a
TRANSPOSED layout [channels(partitions) x sequences(free)]: per step the
xw slice is injected into PSUM via PE transposes (start of the accumulate
group) and the Wh contribution streams as 128x128-stationary matmuls with
N=10 moving columns. Hidden states land directly in resident transposed
h-buffers that feed both the next step's matmuls and the next layer's
input projection as stationary operands.
"""
import sys
import numpy as np

sys.path.insert(0, "/opt/trn_rl_repo")

import ml_dtypes  # noqa: E402
import concourse.bass as bass  # noqa: E402
import concourse.mybir as mybir  # noqa: E402
from concourse import bacc  # noqa: E402
from concourse.tile import TileContext  # noqa: E402
from concourse.masks import make_identity  # noqa: E402
from concourse.bass_utils import run_bass_kernel_spmd  # noqa: E402

F32 = mybir.dt.float32
BF16 = mybir.dt.bfloat16
FP8 = mybir.dt.float8e4
BF16_NP = ml_dtypes.bfloat16
FP8_NP = ml_dtypes.float8_e4m3
DR = mybir.MatmulPerfMode.DoubleRow

DIM = 768
H = 768
G = 4 * H            # 3072, gate order reordered to [i, f, o, g]
B, T = 4, 2000
WIDTH, STRIDE = 200, 100
NFR = 20             # frames per batch element
NSEQ = B * NFR       # 80
NCORES = 8
SEQ_PC = NSEQ // NCORES   # 10
ROWS = SEQ_PC * WIDTH     # 2000 rows per core, row = t*10 + s (t-major)
MT = (ROWS + 127) // 128  # 16 row m-tiles (last has 80 rows)
CH_STEPS = 10             # timesteps per xw chunk DMA
NCH = WIDTH // CH_STEPS   # 20 chunks

SIG = mybir.ActivationFunctionType.Sigmoid
TANH = mybir.ActivationFunctionType.Tanh
MUL = mybir.AluOpType.mult
ADD = mybir.AluOpType.add

_CACHE = {}


def _mrows(m):
    return min(128, ROWS - m * 128)


def _build_program():
    nc = bacc.Bacc("TRN2", target_bir_lowering=False, debug=False,
                   num_devices=NCORES)

    xT_d = nc.declare_dram_parameter("xT", [DIM, ROWS], FP8, isOutput=False)
    wx0_d = nc.declare_dram_parameter("wx0", [2, DIM, G], FP8, isOutput=False)
    wh0_d = nc.declare_dram_parameter("wh0", [2, H, G], BF16, isOutput=False)
    b0_d = nc.declare_dram_parameter("b0", [2, 128, 24], F32, isOutput=False)
    wx1_d = nc.declare_dram_parameter("wx1", [2, 2 * H, G], FP8,
                                      isOutput=False)
    wh1_d = nc.declare_dram_parameter("wh1", [2, H, G], BF16, isOutput=False)
    b1_d = nc.declare_dram_parameter("b1", [2, 128, 24], F32, isOutput=False)
    linw_d = nc.declare_dram_parameter("linw", [2 * H, DIM], BF16,
                                       isOutput=False)
    linb_d = nc.declare_dram_parameter("linb", [DIM], F32, isOutput=False)
    out_d = nc.declare_dram_parameter("out", [ROWS, DIM], F32, isOutput=True)

    # xw stored transposed: [dir, gate m-tile, partition(gate%128), row]
    xw0_d = nc.dram_tensor("xw0", [2, 24, 128, ROWS], BF16)
    xw1_d = nc.dram_tensor("xw1", [2, 24, 128, ROWS], BF16)

    with TileContext(nc) as tc:
        with tc.tile_pool(name="const", bufs=1) as constp:
            identb = constp.tile([128, 128], BF16)
            make_identity(nc, identb[:])
            ones = constp.tile([1, 128], F32)
            nc.vector.memset(ones[:], 1.0)

            # ---- batched input projection, transposed output ----
            # xwT[d, m, p, row] = sum_k rhs_fn(row)[k] * Wx[k, m*128+p] + b
            # fp8 DoubleRow: each matmul consumes two 128-row K-tiles via
            # [128, 2, *] APs on both operands.
            def proj(kt, rhs_fn, wx_dram, b_dram, xw_dram, mid=None,
                     interleave=None):
                k2t = kt // 2
                for d in range(2):
                    if d == 1 and mid is not None:
                        mid()
                    with tc.tile_pool(name="wxp", bufs=1) as wxp, \
                         tc.tile_pool(name="bbp", bufs=1) as bbp, \
                         tc.tile_pool(name="pp", bufs=3, space="PSUM") as pp, \
                         tc.tile_pool(name="xo", bufs=3) as xop:
                        wx_sb = wxp.tile([128, kt, G], FP8)
                        for k in range(kt):
                            nc.sync.dma_start(
                                wx_sb[:, k],
                                wx_dram[d, k * 128:(k + 1) * 128, :])
                            if interleave is not None:
                                interleave(d, k)
                        bT = bbp.tile([128, 24], F32)
                        nc.sync.dma_start(bT[:], b_dram[d])
                        for m in range(24):
                            xo = xop.tile([128, ROWS], BF16, tag="xo")
                            for r in range(4):
                                rs = slice(r * 500, (r + 1) * 500)
                                ps = pp.tile([128, 500], F32, tag="pp")
                                for k2 in range(k2t):
                                    nc.tensor.matmul(
                                        ps[:],
                                        wx_sb[:, 2 * k2:2 * k2 + 2,
                                              m * 128:(m + 1) * 128],
                                        rhs_fn(d, k2, rs),
                                        start=(k2 == 0), stop=(k2 == k2t - 1),
                                        perf_mode=DR)
                                # drain alternates DVE/Act so neither binds
                                if r % 2 == 0:
                                    nc.vector.tensor_scalar(
                                        xo[:, rs], ps[:], bT[:, m:m + 1],
                                        None, ADD)
                                else:
                                    nc.scalar.add(xo[:, rs], ps[:],
                                                  bT[:, m:m + 1])
                            nc.sync.dma_start(xw_dram[d, m], xo[:])

            # ---- recurrence, both dirs interleaved, transposed layout ----
            # Gates split into two PSUM halves: [i,f] (m-tiles 0-11) and
            # [o,g] (m-tiles 12-23), so the sigmoid chain of a step starts
            # after only half its matmuls and hides under the rest.
            def recur(whp, wh_sb, xw_dram, hT, h8=None):
                with tc.tile_pool(name="stp", bufs=1) as stp, \
                     tc.tile_pool(name="xcp", bufs=2) as xcp, \
                     tc.tile_pool(name="gpp", bufs=2, space="PSUM") as gpp, \
                     tc.tile_pool(name="sgp", bufs=2) as sgp:
                    c = [stp.tile([128, 60], F32, name=f"c{d}")
                         for d in range(2)]
                    for d in range(2):
                        nc.vector.memset(c[d][:], 0.0)

                    for ch in range(NCH):
                        xc = []
                        for d in range(2):
                            cs = ch * 100 if d == 0 else 1900 - ch * 100
                            xct = xcp.tile([128, 24, 100], BF16,
                                           tag=f"xc{d}", name=f"xc{d}")
                            nc.sync.dma_start(
                                xct[:],
                                xw_dram[d, :, :, cs:cs + 100].rearrange(
                                    "m p c -> p m c"))
                            xc.append(xct)
                        for tl in range(CH_STEPS):
                            t = ch * CH_STEPS + tl
                            first = (t == 0)
                            sg, tcs = [], []
                            for d in range(2):
                                tt = t if d == 0 else WIDTH - 1 - t
                                lc = tl * 10 if d == 0 else (9 - tl) * 10
                                pv = (t - 1) * 10 if d == 0 else (tt + 1) * 10
                                gif = gpp.tile([128, 120], F32,
                                               tag=f"gi{d}", name=f"gi{d}")
                                gog = gpp.tile([128, 120], F32,
                                               tag=f"go{d}", name=f"go{d}")
                                for m in range(24):
                                    gg = gif if m < 12 else gog
                                    ms = slice((m % 12) * 10,
                                               (m % 12) * 10 + 10)
                                    nc.tensor.matmul(
                                        gg[:, ms],
                                        identb[:],
                                        xc[d][:, m, lc:lc + 10],
                                        start=True, stop=first)
                                    if not first:
                                        for k in range(6):
                                            nc.tensor.matmul(
                                                gg[:, ms],
                                                wh_sb[:, d, k,
                                                      m * 128:(m + 1) * 128],
                                                hT[d][:, k, pv:pv + 10],
                                                start=False, stop=(k == 5))
                                sgif = sgp.tile([128, 120], F32,
                                                tag=f"si{d}", name=f"si{d}")
                                tgg = sgp.tile([128, 60], F32,
                                               tag=f"tg{d}", name=f"tg{d}")
                                sgo = sgp.tile([128, 60], F32,
                                               tag=f"so{d}", name=f"so{d}")
                                tmm = sgp.tile([128, 60], F32,
                                               tag=f"tm{d}", name=f"tm{d}")
                                sg.append((sgif, sgo))
                                tcs.append(sgp.tile([128, 60], F32,
                                                    tag=f"tc{d}",
                                                    name=f"tc{d}"))
                                nc.scalar.activation(sgif[:], gif[:], SIG)
                                nc.scalar.activation(tgg[:], gog[:, 60:120],
                                                     TANH)
                                nc.scalar.activation(sgo[:], gog[:, 0:60],
                                                     SIG)
                                nc.gpsimd.tensor_tensor(
                                    tmm[:], sgif[:, 0:60], tgg[:], MUL)
                                nc.vector.tensor_tensor(
                                    c[d][:], c[d][:], sgif[:, 60:120], MUL)
                                nc.vector.tensor_tensor(
                                    c[d][:], c[d][:], tmm[:], ADD)
                            for d in range(2):
                                tt = t if d == 0 else WIDTH - 1 - t
                                nc.scalar.activation(tcs[d][:], c[d][:], TANH)
                                nc.vector.tensor_tensor(
                                    hT[d][:, :, tt * 10:(tt + 1) * 10],
                                    sg[d][1][:].rearrange(
                                        "p (a b) -> p a b", b=10),
                                    tcs[d][:].rearrange(
                                        "p (a b) -> p a b", b=10),
                                    MUL)
                            if h8 is not None:
                                for d in range(2):
                                    tt = t if d == 0 else WIDTH - 1 - t
                                    nc.gpsimd.tensor_tensor(
                                        h8[d][:, :, tt * 10:(tt + 1) * 10],
                                        sg[d][1][:].rearrange(
                                            "p (a b) -> p a b", b=10),
                                        tcs[d][:].rearrange(
                                            "p (a b) -> p a b", b=10),
                                        MUL)

            # ---- final linear: y = h1cat @ linW + linb ----
            def linear(hT1, lwp, lw, lbsb):
                with tc.tile_pool(name="lpp", bufs=3, space="PSUM") as lpp, \
                     tc.tile_pool(name="lop", bufs=2) as lop:
                    lbb = lwp.tile([128, DIM], F32)
                    for n in range(2):
                        ns = slice(n * 384, (n + 1) * 384)
                        bps = lpp.tile([128, 384], F32, tag="lp")
                        nc.tensor.matmul(bps[:], ones[:], lbsb[:, ns],
                                         start=True, stop=True)
                        nc.vector.tensor_copy(lbb[:, ns], bps[:])
                    for m in range(MT):
                        mr = _mrows(m)
                        lo = lop.tile([128, DIM], F32, tag="lo")
                        for n in range(2):
                            ns = slice(n * 384, (n + 1) * 384)
                            ps = lpp.tile([mr, 384], F32, tag="lp")
                            for k in range(12):
                                ht = hT1[0] if k < 6 else hT1[1]
                                kk = k if k < 6 else k - 6
                                nc.tensor.matmul(
                                    ps[:],
                                    ht[:, kk, m * 128:m * 128 + mr],
                                    lw[:, k, ns],
                                    start=(k == 0), stop=(k == 11))
                            nc.vector.tensor_tensor(
                                lo[:mr, ns], ps[:], lbb[:mr, ns], ADD)
                        nc.sync.dma_start(out_d[m * 128:m * 128 + mr, :],
                                          lo[:mr])

            def load_wh(whp, wh_dram, name):
                wh_sb = whp.tile([128, 2, 6, G], BF16, name=name)
                for d in range(2):
                    for k in range(6):
                        nc.sync.dma_start(
                            wh_sb[:, d, k],
                            wh_dram[d, k * 128:(k + 1) * 128, :])
                return wh_sb

            # h0/h1 share 2 slots: h1 reuses h0's space after proj1.
            with tc.tile_pool(name="hbp", bufs=2) as hbp, \
                 tc.tile_pool(name="h8p", bufs=1) as h8p:
                hT0 = [hbp.tile([128, 6, ROWS], BF16, tag="hb",
                                name=f"h0{d}") for d in range(2)]
                hT1 = [hbp.tile([128, 6, ROWS], BF16, tag="hb",
                                name=f"h1{d}") for d in range(2)]
                h08 = [h8p.tile([128, 6, ROWS], FP8, name=f"h08{d}")
                       for d in range(2)]
                with tc.tile_pool(name="whp0", bufs=1) as whp0:
                    wh0_box = []
                    with tc.tile_pool(name="xtp", bufs=1) as xtp:
                        xT_sb = xtp.tile([128, 6, ROWS], FP8)
                        proj(6,
                             lambda d, k2, rs:
                             xT_sb[:, 2 * k2:2 * k2 + 2, rs],
                             wx0_d, b0_d, xw0_d,
                             mid=lambda: wh0_box.append(
                                 load_wh(whp0, wh0_d, "wh0")),
                             interleave=lambda d, k:
                             nc.sync.dma_start(
                                 xT_sb[:, k],
                                 xT_d[k * 128:(k + 1) * 128, :])
                             if d == 0 else None)
                    recur(whp0, wh0_box[0], xw0_d, hT0, h8=h08)
                with tc.tile_pool(name="whp1", bufs=1) as whp1:
                    wh1_box = []
                    proj(12,
                         lambda d, k2, rs:
                         h08[0][:, 2 * k2:2 * k2 + 2, rs] if k2 < 3
                         else h08[1][:, 2 * k2 - 6:2 * k2 - 4, rs],
                         wx1_d, b1_d, xw1_d,
                         mid=lambda: wh1_box.append(
                             load_wh(whp1, wh1_d, "wh1")))
                    with tc.tile_pool(name="lwp", bufs=1) as lwp:
                        lw = lwp.tile([128, 12, DIM], BF16)
                        for k in range(12):
                            nc.sync.dma_start(
                                lw[:, k], linw_d[k * 128:(k + 1) * 128, :])
                        lbsb = lwp.tile([1, DIM], F32)
                        nc.sync.dma_start(lbsb[:], linb_d[None, :])
                        recur(whp1, wh1_box[0], xw1_d, hT1)
                        linear(hT1, lwp, lw, lbsb)

    nc.compile()
    return nc


def _reorder_gates(w):
    """[i f g o] -> [i f o g] along last axis (size 4H)."""
    i, f, g, o = np.split(w, 4, axis=-1)
    return np.concatenate([i, f, o, g], axis=-1)


def kernel(x, Wx0f, Wh0f, b0f, Wx0b, Wh0b, b0b,
           Wx1f, Wh1f, b1f, Wx1b, Wh1b, b1b, lin_W, lin_b):
    x = np.asarray(x, dtype=np.float32)
    # frame: (B, C, T) -> (NSEQ, DIM, WIDTH)
    tgt = (NFR - 1) * STRIDE + WIDTH
    xp = np.zeros((B, DIM, tgt), dtype=np.float32)
    xp[:, :, :T] = x
    frames = np.stack([xp[:, :, i:i + WIDTH]
                       for i in range(0, tgt - WIDTH + 1, STRIDE)], axis=1)
    xf = frames.reshape(NSEQ, DIM, WIDTH)

    def prepw(wf, wb, dt=BF16_NP):
        return np.ascontiguousarray(np.stack(
            [_reorder_gates(np.asarray(wf, np.float32)),
             _reorder_gates(np.asarray(wb, np.float32))])).astype(dt)

    def prepb(bf, bb_):
        # transposed bias: [dir, partition(gate%128), gate m-tile]
        return np.ascontiguousarray(np.stack(
            [_reorder_gates(np.asarray(bf, np.float32)).reshape(24, 128).T,
             _reorder_gates(np.asarray(bb_, np.float32)).reshape(24, 128).T]))

    wx0 = prepw(Wx0f, Wx0b, FP8_NP)
    wh0 = prepw(Wh0f, Wh0b)
    b0 = prepb(b0f, b0b)
    wx1 = prepw(Wx1f, Wx1b, FP8_NP)
    wh1 = prepw(Wh1f, Wh1b)
    b1 = prepb(b1f, b1b)
    linw = np.ascontiguousarray(np.asarray(lin_W, np.float32)).astype(BF16_NP)
    linb = np.ascontiguousarray(np.asarray(lin_b, np.float32))

    if "nc" not in _CACHE:
        _CACHE["nc"] = _build_program()
    nc = _CACHE["nc"]

    in_maps = []
    for cc in range(NCORES):
        shard = xf[cc * SEQ_PC:(cc + 1) * SEQ_PC]       # (10, 768, 200)
        xT = shard.transpose(1, 2, 0).reshape(DIM, ROWS)  # col = t*10 + s
        in_maps.append({"xT": np.ascontiguousarray(xT).astype(FP8_NP),
                        "wx0": wx0, "wh0": wh0, "b0": b0,
                        "wx1": wx1, "wh1": wh1, "b1": b1,
                        "linw": linw, "linb": linb})
    _CACHE["in_maps"] = in_maps

    res = run_bass_kernel_spmd(nc, in_maps, list(range(NCORES)))
    outs = [np.asarray(res.results[cc]["out"], np.float32)
            .reshape(WIDTH, SEQ_PC, DIM).transpose(1, 0, 2)
            for cc in range(NCORES)]                     # (10, 200, 768)
    y = np.concatenate(outs, axis=0)                     # (80, 200, 768)
    y = y.transpose(0, 2, 1).reshape(B, NFR, DIM, WIDTH)

    limit = STRIDE // 2
    parts = [y[:, 0, :, :-limit]]
    for k in range(1, NFR - 1):
        parts.append(y[:, k, :, limit:-limit])
    parts.append(y[:, NFR - 1, :, limit:])
    yc = np.concatenate(parts, axis=-1)[:, :, :T]        # (4, 768, 2000)
    return (yc + x).astype(np.float32)
